# revision 23
# baseline (speedup 1.0000x reference)
"""GAT single-head forward on 8 Trainium2 NeuronCores (Bass/Tile).

Math (per reference):
    h   = X @ W + b                      [N, 128]
    f1  = h @ v0, f2 = h @ v1            [N]
    logits = adj * (f1[:,None] + f2[None,:])   (adj entries are exactly 0/1)
    vals = sigmoid(logits) - 0.5
    masked softmax over row edges; out = probs @ h

Key identities used on device:
  * On edges (adj==1): val = sigmoid(f1_i + f2_j) - 0.5 in (-0.5, 0.5), so the
    softmax max-subtraction is unnecessary (exp never overflows) and constant
    shifts cancel:  probs = adj*exp(sigmoid(s)) / rowsum(adj*exp(sigmoid(s))).
  * sigmoid(s) = 0.5*tanh(s/2) + 0.5, so exp(sigmoid(s)) = exp(0.5*t + 0.5)
    with t = tanh(s/2). Tanh and Exp live in the SAME activation table set
    ("exp_and_others"), avoiding per-tile ~2.7us table swaps that
    Sigmoid+Exp would incur.
  * A ones-column appended to h turns the softmax denominator into one extra
    matmul output column (no separate row-reduction pass).

Sharding: rows of adj across the 8 cores (1024 rows each). node_feats is
small (8 MB) and is replicated, so every core computes the full projected
h locally - no collectives at all.

Per-core layout trick: each core works on its adj block TRANSPOSED
([j=source node on partitions, i=own rows on free dim]) so that the
aggregate probs@h contracts over the partition dim as the tensor engine
requires. The transpose rides the DMA x-bar (free at ~261GB/s); adj is cast
to fp16 host-side (exact for a 0/1 mask, and halves HBM traffic).

The projection inputs are fed in fp16 (validated 4.2e-4 scale-relative
absmax on the final output): fp16 matmuls stream 2x faster and get fast
weight loads, and the w0/w1 columns are pre-halved on the host so the
tanh argument s/2 comes straight out of the projection matmul.
"""

import os

import numpy as np

import concourse.bass as bass
import concourse.mybir as mybir
import concourse.tile as tile
from concourse import bacc
from concourse.bass_utils import run_bass_kernel_spmd

F32 = mybir.dt.float32
F16 = mybir.dt.float16
AF = mybir.ActivationFunctionType

N, C_IN, C_OUT = 8192, 256, 128
NCORES = 8
ROWS = N // NCORES          # 1024 rows of adj per core
P = 128
NT = N // P                 # 64 node tiles (also the j-chunks)
NI = ROWS // P              # 8 output row-tiles per core
GROUP = 8                   # j-chunks fused per activation instruction
NGROUPS = NT // GROUP
KC = [128, 128, 1]          # contraction chunks of K=257 (X.T rows + ones row)
# activation groups: j-chunks fused per tanh/exp instruction; the final 8
# are split 4+4 so the post-exp tail (mask-mul + matmul + epilogue) is short
GROUPS = [8] * 7 + [4, 4]
WCOLS = C_OUT + 3           # [W | ones-hack | 0.5*w0 | 0.5*w1]
HCOLS = C_OUT + 1           # h plus the ones column
TINY = float(np.finfo(np.float32).tiny)

_CACHE: dict = {}


def _build_nc(b_zero=True):
    nc = bacc.Bacc(
        "TRN2", target_bir_lowering=False, debug=False, num_devices=NCORES
    )
    xt1 = nc.dram_tensor("xt1", [257, N], F16, kind="ExternalInput").ap()
    xt1l = nc.dram_tensor("xt1l", [257, ROWS], F16, kind="ExternalInput").ap()
    wext = nc.dram_tensor("wext", [257, WCOLS], F16, kind="ExternalInput").ap()
    adj = nc.dram_tensor("adj", [ROWS, N], F16, kind="ExternalInput").ap()
    out = nc.dram_tensor("out", [ROWS, C_OUT], F32, kind="ExternalOutput").ap()

    with tile.TileContext(nc) as tc:
        _emit(tc, nc, xt1, xt1l, wext, adj, out, b_zero)
    nc.compile()
    return nc


def _emit(tc, nc, xt1, xt1l, wext, adj, out, b_zero):
    # with b == 0 the K=1 "ones row" contraction chunk only contributes the
    # constant-one column of h_ext (done with a strided memset instead) and
    # zero constants to f1/f2 -- skip it entirely.
    nkc = 2 if b_zero else 3
    from contextlib import ExitStack

    with ExitStack() as ctx:
        # ---- persistent tiles for the main loop ----
        persist = ctx.enter_context(tc.tile_pool(name="persist", bufs=1))
        h16_all = persist.tile([P, NT * HCOLS], F16, tag="h16")   # [128, 8256]
        f2h_all = persist.tile([P, NT], F32, tag="f2h")           # 0.5*f2 per j
        f1rep = persist.tile([P, ROWS], F16, tag="f1rep")         # f1/2 bcast
        ones_row = persist.tile([1, P], F16, tag="ones")
        nc.vector.memset(ones_row[:], 1.0)
        zero1 = persist.tile([P, 1], F32, tag="zero1")
        nc.vector.memset(zero1[:], 0.0)
        half1 = persist.tile([P, 1], F32, tag="half1")
        nc.vector.memset(half1[:], 0.5)

        with ExitStack() as phase_a:
            xtp = phase_a.enter_context(tc.tile_pool(name="xt", bufs=1))
            fsb = phase_a.enter_context(tc.tile_pool(name="fsb", bufs=1))

            # small inputs first so the f1 path clears quickly; loads ride
            # the scalar HWDGE ring so the sync ring starts the adj
            # transposes immediately
            wes, xls = [], []
            off = 0
            for k in range(nkc):
                kc = KC[k]
                we_sb = xtp.tile([kc, WCOLS], F16, tag=f"we{k}")
                nc.scalar.dma_start(we_sb[:], wext[off : off + kc, :])
                xl_sb = xtp.tile([kc, ROWS], F16, tag=f"xl{k}")
                nc.scalar.dma_start(xl_sb[:], xt1l[off : off + kc, :])
                wes.append(we_sb)
                xls.append(xl_sb)
                off += kc
            xts = []
            off = 0
            for k in range(nkc):
                kc = KC[k]
                xt_sb = xtp.tile([kc, N], F16, tag=f"xt{k}")
                nsub = 4 if kc == P else 1
                sub = N // nsub
                for c in range(nsub):
                    nc.scalar.dma_start(
                        xt_sb[:, c * sub : (c + 1) * sub],
                        xt1[off : off + kc, c * sub : (c + 1) * sub],
                    )
                xts.append(xt_sb)
                off += kc

            # f1/2 for this core's own rows, as a row vector [1, 1024]
            with tc.tile_pool(name="pf", bufs=1, space="PSUM") as pfp:
                pf1 = pfp.tile([1, ROWS], F32, tag="pf1")
                for nh in range(ROWS // 512):
                    for k in range(nkc):
                        nc.tensor.matmul(
                            pf1[:, nh * 512 : (nh + 1) * 512],
                            wes[k][:, C_OUT + 1 : C_OUT + 2],
                            xls[k][:, nh * 512 : (nh + 1) * 512],
                            start=(k == 0),
                            stop=(k == nkc - 1),
                        )
                f1row = fsb.tile([1, ROWS], F16, tag="f1row")
                nc.vector.tensor_copy(f1row[:], pf1[:])

                # broadcast across partitions via a K=1 matmul w/ ones column
                prep = pfp.tile([P, ROWS], F32, tag="prep")
                for nh in range(ROWS // 512):
                    sl = slice(nh * 512, (nh + 1) * 512)
                    nc.tensor.matmul(
                        prep[:, sl], ones_row[:], f1row[:, sl],
                        start=True, stop=True,
                    )
                nc.vector.tensor_copy(f1rep[:], prep[:])

            # h-projection: all 8 PSUM banks inside ONE tensor so four tiles
            # drain with a single strided copy. Pairs of node tiles have
            # their k-chunk matmuls interleaved so consecutive matmuls hit
            # different banks (same-bank accumulation serializes the PE).
            BANK = 512
            # constant-one column of every h_ext tile (replaces the K=1
            # bias matmul chunk when b == 0)
            if b_zero:
                nc.vector.memset(
                    h16_all[:].rearrange("p (t c) -> p t c", c=HCOLS)[
                        :, :, C_OUT : C_OUT + 1
                    ],
                    1.0,
                )
            with tc.tile_pool(name="php", bufs=1, space="PSUM") as php:
                ph_all = php.tile([P, NI * BANK], F32, tag="ph")
                for nt0 in range(0, NT, 2):
                    w0 = (nt0 % NI) * BANK
                    w1 = ((nt0 + 1) % NI) * BANK
                    for k in range(nkc):
                        nc.tensor.matmul(
                            ph_all[:, w0 : w0 + WCOLS],
                            xts[k][:, nt0 * P : (nt0 + 1) * P],
                            wes[k][:],
                            start=(k == 0),
                            stop=(k == nkc - 1),
                        )
                        nc.tensor.matmul(
                            ph_all[:, w1 : w1 + WCOLS],
                            xts[k][:, (nt0 + 1) * P : (nt0 + 2) * P],
                            wes[k][:],
                            start=(k == 0),
                            stop=(k == nkc - 1),
                        )
                    if nt0 % 4 == 2:
                        # drain the 4 freshly produced tiles in two strided
                        # copies: h||ones -> fp16, and 0.5*f2 columns
                        b = nt0 - 2  # first of the 4 node tiles
                        half = (b % NI) * BANK
                        src = ph_all[:, half : half + 4 * BANK].rearrange(
                            "p (b w) -> p b w", b=4
                        )
                        dst_h = h16_all[
                            :, b * HCOLS : (b + 4) * HCOLS
                        ].rearrange("p (b w) -> p b w", b=4)
                        hc = C_OUT if b_zero else HCOLS
                        nc.vector.tensor_copy(
                            dst_h[:, :, 0:hc], src[:, :, 0:hc]
                        )
                        nc.vector.tensor_copy(
                            f2h_all[:, b : b + 4],
                            src[:, :, C_OUT + 2 : C_OUT + 3],
                        )

            # ================= main loop =================
            pop = phase_a.enter_context(
                tc.tile_pool(name="po", bufs=1, space="PSUM")
            )
            pouts = [
                pop.tile([P, HCOLS], F32, name=f"po{i}", tag=f"po{i}")
                for i in range(NI)
            ]
            sup = phase_a.enter_context(tc.tile_pool(name="sup", bufs=2))
            g16p = phase_a.enter_context(tc.tile_pool(name="g16p", bufs=2))
            atp = phase_a.enter_context(
                tc.tile_pool(name="atp", bufs=2 * GROUP)
            )
            etp = phase_a.enter_context(tc.tile_pool(name="etp", bufs=10))
            obp = phase_a.enter_context(tc.tile_pool(name="ob", bufs=4))

            q0 = 0
            for g, gsz in enumerate(GROUPS):
                s_sup = sup.tile([P, gsz * ROWS], F16, tag="s", bufs=2)
                t_sup = sup.tile([P, gsz * ROWS], F16, tag="t", bufs=1)
                g16 = g16p.tile([P, gsz * ROWS], F16, tag="g16")
                ats = []
                for qq in range(gsz):
                    q = q0 + qq
                    at = atp.tile([P, ROWS], F16, tag="at")
                    nc.sync.dma_start(
                        at[:], adj[:, q * P : (q + 1) * P], transpose=True
                    )
                    ats.append(at)
                    # s = 0.5*f1_i + 0.5*f2_j   [j on partitions, i on free]
                    nc.vector.tensor_scalar_add(
                        s_sup[:, qq * ROWS : (qq + 1) * ROWS],
                        f1rep[:],
                        f2h_all[:, q : q + 1],
                    )
                # one fused tanh + one fused exp over the whole group
                nc.scalar.activation(t_sup[:], s_sup[:], AF.Tanh, bias=zero1[:])
                # exp(0.5*tanh + 0.5) = exp(sigmoid(s)), output fp16
                nc.scalar.activation(
                    g16[:], t_sup[:], AF.Exp, bias=half1[:], scale=0.5
                )

                for qq in range(gsz):
                    q = q0 + qq
                    et = etp.tile([P, ROWS], F16, tag="et")
                    nc.vector.tensor_mul(
                        et[:], ats[qq][:], g16[:, qq * ROWS : (qq + 1) * ROWS]
                    )
                    rhs = h16_all[:, q * HCOLS : (q + 1) * HCOLS]
                    for it in range(NI):
                        nc.tensor.matmul(
                            pouts[it][:],
                            et[:, it * P : (it + 1) * P],
                            rhs,
                            start=(q == 0),
                            stop=(q == NT - 1),
                        )
                q0 += gsz

            # ================= epilogue =================
            for it in range(NI):
                po = pouts[it]
                dm = obp.tile([P, 1], F32, tag="dm")
                nc.vector.tensor_scalar_max(
                    dm[:], po[:, C_OUT : C_OUT + 1], TINY
                )
                rc = obp.tile([P, 1], F32, tag="rc")
                nc.vector.reciprocal(rc[:], dm[:])
                ob = obp.tile([P, C_OUT], F32, tag="ob")
                nc.vector.tensor_scalar_mul(ob[:], po[:, 0:C_OUT], rc[:])
                nc.sync.dma_start(out[it * P : (it + 1) * P, :], ob[:])


def _prep_inputs(node_feats, adj_matrix, W, b, v0, v1):
    X = np.ascontiguousarray(node_feats, dtype=np.float32)
    W = np.asarray(W, dtype=np.float32)
    b = np.asarray(b, dtype=np.float32)
    v0 = np.asarray(v0, dtype=np.float32)
    v1 = np.asarray(v1, dtype=np.float32)

    w0h = (0.5 * (W.astype(np.float64) @ v0.astype(np.float64))).astype(np.float32)
    w1h = (0.5 * (W.astype(np.float64) @ v1.astype(np.float64))).astype(np.float32)
    c0h = np.float32(0.5 * float(b.astype(np.float64) @ v0.astype(np.float64)))
    c1h = np.float32(0.5 * float(b.astype(np.float64) @ v1.astype(np.float64)))

    XT1 = np.empty((257, N), np.float32)
    XT1[:256] = X.T
    XT1[256] = 1.0

    WE = np.zeros((257, WCOLS), np.float32)
    WE[:256, :C_OUT] = W
    WE[256, :C_OUT] = b
    WE[256, C_OUT] = 1.0          # makes h_ext column 128 identically 1
    WE[:256, C_OUT + 1] = w0h
    WE[256, C_OUT + 1] = c0h
    WE[:256, C_OUT + 2] = w1h
    WE[256, C_OUT + 2] = c1h

    XT1h = XT1.astype(np.float16)
    WEh = WE.astype(np.float16)
    A16 = np.asarray(adj_matrix, dtype=np.float16)

    in_maps = []
    for c in range(NCORES):
        in_maps.append(
            {
                "xt1": XT1h,
                "xt1l": np.ascontiguousarray(XT1h[:, c * ROWS : (c + 1) * ROWS]),
                "wext": WEh,
                "adj": np.ascontiguousarray(A16[c * ROWS : (c + 1) * ROWS, :]),
            }
        )
    return in_maps


def _run(in_maps, trace=False, b_zero=True):
    key = f"nc_b{int(b_zero)}"
    if key not in _CACHE:
        _CACHE[key] = _build_nc(b_zero=b_zero)
    nc = _CACHE[key]
    res = run_bass_kernel_spmd(
        nc, in_maps, core_ids=list(range(NCORES)), trace=trace
    )
    full = np.concatenate(
        [res.results[c]["out"] for c in range(NCORES)], axis=0
    ).astype(np.float32)
    return full, res


def kernel(node_feats, adj_matrix, W, b, v0, v1):
    in_maps = _prep_inputs(node_feats, adj_matrix, W, b, v0, v1)
    trace = bool(int(os.environ.get("GAT_TRACE", "0")))
    b_zero = not bool(np.any(np.asarray(b)))
    full, _ = _run(in_maps, trace=trace, b_zero=b_zero)
    return full


# revision 25
# speedup vs baseline: 1.4384x; 1.4384x over previous
"""GAT single-head forward on 8 Trainium2 NeuronCores (Bass/Tile).

Math (per reference):
    h   = X @ W + b                      [N, 128]
    f1  = h @ v0, f2 = h @ v1            [N]
    logits = adj * (f1[:,None] + f2[None,:])   (adj entries are exactly 0/1)
    vals = sigmoid(logits) - 0.5
    masked softmax over row edges; out = probs @ h

Key identities used on device:
  * On edges (adj==1): val = sigmoid(f1_i + f2_j) - 0.5 in (-0.5, 0.5), so the
    softmax max-subtraction is unnecessary (exp never overflows) and constant
    shifts cancel:  probs = adj*exp(sigmoid(s)) / rowsum(adj*exp(sigmoid(s))).
  * sigmoid(s) = 0.5*tanh(s/2) + 0.5, so exp(sigmoid(s)) = exp(0.5*t + 0.5)
    with t = tanh(s/2). Tanh and Exp live in the SAME activation table set
    ("exp_and_others"), avoiding per-tile ~2.7us table swaps that
    Sigmoid+Exp would incur.
  * A ones-column appended to h turns the softmax denominator into one extra
    matmul output column (no separate row-reduction pass).

Sharding: rows of adj across the 8 cores (1024 rows each). node_feats is
small (8 MB) and is replicated, so every core computes the full projected
h locally - no collectives at all.

Per-core layout trick: each core works on its adj block TRANSPOSED
([j=source node on partitions, i=own rows on free dim]) so that the
aggregate probs@h contracts over the partition dim as the tensor engine
requires. The transpose rides the DMA x-bar (free at ~261GB/s); adj is cast
to fp16 host-side (exact for a 0/1 mask, and halves HBM traffic).

The projection inputs are fed in fp16 (validated 4.2e-4 scale-relative
absmax on the final output): fp16 matmuls stream 2x faster and get fast
weight loads, and the w0/w1 columns are pre-halved on the host so the
tanh argument s/2 comes straight out of the projection matmul.
"""

import os

import numpy as np

import concourse.bass as bass
import concourse.mybir as mybir
import concourse.tile as tile
from concourse import bacc
from concourse.bass_utils import run_bass_kernel_spmd

F32 = mybir.dt.float32
F16 = mybir.dt.float16
AF = mybir.ActivationFunctionType

N, C_IN, C_OUT = 8192, 256, 128
NCORES = 8
ROWS = N // NCORES          # 1024 rows of adj per core
P = 128
NT = N // P                 # 64 node tiles (also the j-chunks)
NI = ROWS // P              # 8 output row-tiles per core
GROUP = 8                   # j-chunks fused per activation instruction
NGROUPS = NT // GROUP
KC = [128, 128, 1]          # contraction chunks of K=257 (X.T rows + ones row)
# activation groups: j-chunks fused per tanh/exp instruction; the final 8
# are split 4+4 so the post-exp tail (mask-mul + matmul + epilogue) is short
GROUPS = [8] * 7 + [4, 4]
WCOLS = C_OUT + 3           # [W | ones-hack | 0.5*w0 | 0.5*w1]
HCOLS = C_OUT + 1           # h plus the ones column
TINY = float(np.finfo(np.float32).tiny)

_CACHE: dict = {}


def _build_nc(b_zero=True):
    nc = bacc.Bacc(
        "TRN2", target_bir_lowering=False, debug=False, num_devices=NCORES
    )
    xt1 = nc.dram_tensor("xt1", [257, N], F16, kind="ExternalInput").ap()
    xt1l = nc.dram_tensor("xt1l", [257, ROWS], F16, kind="ExternalInput").ap()
    wext = nc.dram_tensor("wext", [257, WCOLS], F16, kind="ExternalInput").ap()
    adj = nc.dram_tensor("adj", [ROWS, N], F16, kind="ExternalInput").ap()
    out = nc.dram_tensor("out", [ROWS, C_OUT], F32, kind="ExternalOutput").ap()

    with tile.TileContext(nc) as tc:
        _emit(tc, nc, xt1, xt1l, wext, adj, out, b_zero)
    nc.compile()
    return nc


def _emit(tc, nc, xt1, xt1l, wext, adj, out, b_zero):
    # with b == 0 the K=1 "ones row" contraction chunk only contributes the
    # constant-one column of h_ext (done with a strided memset instead) and
    # zero constants to f1/f2 -- skip it entirely.
    nkc = 2 if b_zero else 3
    from contextlib import ExitStack

    with ExitStack() as ctx:
        # ---- persistent tiles for the main loop ----
        persist = ctx.enter_context(tc.tile_pool(name="persist", bufs=1))
        h16_all = persist.tile([P, NT * HCOLS], F16, tag="h16")   # [128, 8256]
        f2h_all = persist.tile([P, NT], F32, tag="f2h")           # 0.5*f2 per j
        f1rep = persist.tile([P, ROWS], F16, tag="f1rep")         # f1/2 bcast
        ones_row = persist.tile([1, P], F16, tag="ones")
        nc.vector.memset(ones_row[:], 1.0)
        zero1 = persist.tile([P, 1], F32, tag="zero1")
        nc.vector.memset(zero1[:], 0.0)
        half1 = persist.tile([P, 1], F32, tag="half1")
        nc.vector.memset(half1[:], 0.5)

        with ExitStack() as phase_a:
            xtp = phase_a.enter_context(tc.tile_pool(name="xt", bufs=1))
            fsb = phase_a.enter_context(tc.tile_pool(name="fsb", bufs=1))

            # small inputs first so the f1 path clears quickly. The xt
            # sub-loads are interleaved k0/k1 so the first node tiles have
            # BOTH contraction chunks resident as early as possible (tile
            # dependency tracking is AP-range based).
            wes, xls = [], []
            off = 0
            for k in range(nkc):
                kc = KC[k]
                we_sb = xtp.tile([kc, WCOLS], F16, tag=f"we{k}")
                nc.sync.dma_start(we_sb[:], wext[off : off + kc, :])
                xl_sb = xtp.tile([kc, ROWS], F16, tag=f"xl{k}")
                nc.sync.dma_start(xl_sb[:], xt1l[off : off + kc, :])
                wes.append(we_sb)
                xls.append(xl_sb)
                off += kc
            offs = [0, 128, 256]
            xts = [
                xtp.tile([KC[k], N], F16, name=f"xtsb{k}", tag=f"xt{k}")
                for k in range(nkc)
            ]
            NSUB = 4
            sub = N // NSUB
            for c in range(NSUB):
                for k in range(nkc):
                    if KC[k] != P:
                        if c == 0:
                            nc.sync.dma_start(
                                xts[k][:], xt1[offs[k] : offs[k] + KC[k], :]
                            )
                        continue
                    nc.sync.dma_start(
                        xts[k][:, c * sub : (c + 1) * sub],
                        xt1[
                            offs[k] : offs[k] + KC[k],
                            c * sub : (c + 1) * sub,
                        ],
                    )

            # f1/2 for this core's own rows, as a row vector [1, 1024]
            with tc.tile_pool(name="pf", bufs=1, space="PSUM") as pfp:
                pf1 = pfp.tile([1, ROWS], F32, tag="pf1")
                for nh in range(ROWS // 512):
                    for k in range(nkc):
                        nc.tensor.matmul(
                            pf1[:, nh * 512 : (nh + 1) * 512],
                            wes[k][:, C_OUT + 1 : C_OUT + 2],
                            xls[k][:, nh * 512 : (nh + 1) * 512],
                            start=(k == 0),
                            stop=(k == nkc - 1),
                        )
                f1row = fsb.tile([1, ROWS], F16, tag="f1row")
                nc.vector.tensor_copy(f1row[:], pf1[:])

                # broadcast across partitions via a K=1 matmul w/ ones column
                prep = pfp.tile([P, ROWS], F32, tag="prep")
                for nh in range(ROWS // 512):
                    sl = slice(nh * 512, (nh + 1) * 512)
                    nc.tensor.matmul(
                        prep[:, sl], ones_row[:], f1row[:, sl],
                        start=True, stop=True,
                    )
                nc.vector.tensor_copy(f1rep[:], prep[:])

            # h-projection: all 8 PSUM banks inside ONE tensor so four tiles
            # drain with a single strided copy. Pairs of node tiles have
            # their k-chunk matmuls interleaved so consecutive matmuls hit
            # different banks (same-bank accumulation serializes the PE).
            BANK = 512
            # constant-one column of every h_ext tile (replaces the K=1
            # bias matmul chunk when b == 0)
            if b_zero:
                nc.vector.memset(
                    h16_all[:].rearrange("p (t c) -> p t c", c=HCOLS)[
                        :, :, C_OUT : C_OUT + 1
                    ],
                    1.0,
                )
            with tc.tile_pool(name="php", bufs=1, space="PSUM") as php:
                ph_all = php.tile([P, NI * BANK], F32, tag="ph")
                for nt0 in range(0, NT, 2):
                    w0 = (nt0 % NI) * BANK
                    w1 = ((nt0 + 1) % NI) * BANK
                    for k in range(nkc):
                        nc.tensor.matmul(
                            ph_all[:, w0 : w0 + WCOLS],
                            xts[k][:, nt0 * P : (nt0 + 1) * P],
                            wes[k][:],
                            start=(k == 0),
                            stop=(k == nkc - 1),
                        )
                        nc.tensor.matmul(
                            ph_all[:, w1 : w1 + WCOLS],
                            xts[k][:, (nt0 + 1) * P : (nt0 + 2) * P],
                            wes[k][:],
                            start=(k == 0),
                            stop=(k == nkc - 1),
                        )
                    if nt0 % 4 == 2:
                        # drain the 4 freshly produced tiles in two strided
                        # copies: h||ones -> fp16, and 0.5*f2 columns
                        b = nt0 - 2  # first of the 4 node tiles
                        half = (b % NI) * BANK
                        src = ph_all[:, half : half + 4 * BANK].rearrange(
                            "p (b w) -> p b w", b=4
                        )
                        dst_h = h16_all[
                            :, b * HCOLS : (b + 4) * HCOLS
                        ].rearrange("p (b w) -> p b w", b=4)
                        hc = C_OUT if b_zero else HCOLS
                        nc.vector.tensor_copy(
                            dst_h[:, :, 0:hc], src[:, :, 0:hc]
                        )
                        nc.vector.tensor_copy(
                            f2h_all[:, b : b + 4],
                            src[:, :, C_OUT + 2 : C_OUT + 3],
                        )

            # ================= main loop =================
            pop = phase_a.enter_context(
                tc.tile_pool(name="po", bufs=1, space="PSUM")
            )
            pouts = [
                pop.tile([P, HCOLS], F32, name=f"po{i}", tag=f"po{i}")
                for i in range(NI)
            ]
            sup = phase_a.enter_context(tc.tile_pool(name="sup", bufs=2))
            g16p = phase_a.enter_context(tc.tile_pool(name="g16p", bufs=2))
            atp = phase_a.enter_context(
                tc.tile_pool(name="atp", bufs=2 * GROUP)
            )
            etp = phase_a.enter_context(tc.tile_pool(name="etp", bufs=10))
            obp = phase_a.enter_context(tc.tile_pool(name="ob", bufs=4))

            q0 = 0
            for g, gsz in enumerate(GROUPS):
                s_sup = sup.tile([P, gsz * ROWS], F16, tag="s", bufs=2)
                t_sup = sup.tile([P, gsz * ROWS], F16, tag="t", bufs=1)
                g16 = g16p.tile([P, gsz * ROWS], F16, tag="g16")
                ats = []
                for qq in range(gsz):
                    q = q0 + qq
                    at = atp.tile([P, ROWS], F16, tag="at")
                    nc.sync.dma_start(
                        at[:], adj[:, q * P : (q + 1) * P], transpose=True
                    )
                    ats.append(at)
                    # s = 0.5*f1_i + 0.5*f2_j   [j on partitions, i on free]
                    nc.vector.tensor_scalar_add(
                        s_sup[:, qq * ROWS : (qq + 1) * ROWS],
                        f1rep[:],
                        f2h_all[:, q : q + 1],
                    )
                # one fused tanh + one fused exp over the whole group
                nc.scalar.activation(t_sup[:], s_sup[:], AF.Tanh, bias=zero1[:])
                # exp(0.5*tanh + 0.5) = exp(sigmoid(s)), output fp16
                nc.scalar.activation(
                    g16[:], t_sup[:], AF.Exp, bias=half1[:], scale=0.5
                )

                for qq in range(gsz):
                    q = q0 + qq
                    et = etp.tile([P, ROWS], F16, tag="et")
                    nc.vector.tensor_mul(
                        et[:], ats[qq][:], g16[:, qq * ROWS : (qq + 1) * ROWS]
                    )
                    rhs = h16_all[:, q * HCOLS : (q + 1) * HCOLS]
                    for it in range(NI):
                        nc.tensor.matmul(
                            pouts[it][:],
                            et[:, it * P : (it + 1) * P],
                            rhs,
                            start=(q == 0),
                            stop=(q == NT - 1),
                        )
                q0 += gsz

            # ================= epilogue =================
            for it in range(NI):
                po = pouts[it]
                dm = obp.tile([P, 1], F32, tag="dm")
                nc.vector.tensor_scalar_max(
                    dm[:], po[:, C_OUT : C_OUT + 1], TINY
                )
                rc = obp.tile([P, 1], F32, tag="rc")
                nc.vector.reciprocal(rc[:], dm[:])
                ob = obp.tile([P, C_OUT], F32, tag="ob")
                nc.vector.tensor_scalar_mul(ob[:], po[:, 0:C_OUT], rc[:])
                nc.sync.dma_start(out[it * P : (it + 1) * P, :], ob[:])


def _prep_inputs(node_feats, adj_matrix, W, b, v0, v1):
    X = np.ascontiguousarray(node_feats, dtype=np.float32)
    W = np.asarray(W, dtype=np.float32)
    b = np.asarray(b, dtype=np.float32)
    v0 = np.asarray(v0, dtype=np.float32)
    v1 = np.asarray(v1, dtype=np.float32)

    w0h = (0.5 * (W.astype(np.float64) @ v0.astype(np.float64))).astype(np.float32)
    w1h = (0.5 * (W.astype(np.float64) @ v1.astype(np.float64))).astype(np.float32)
    c0h = np.float32(0.5 * float(b.astype(np.float64) @ v0.astype(np.float64)))
    c1h = np.float32(0.5 * float(b.astype(np.float64) @ v1.astype(np.float64)))

    XT1 = np.empty((257, N), np.float32)
    XT1[:256] = X.T
    XT1[256] = 1.0

    WE = np.zeros((257, WCOLS), np.float32)
    WE[:256, :C_OUT] = W
    WE[256, :C_OUT] = b
    WE[256, C_OUT] = 1.0          # makes h_ext column 128 identically 1
    WE[:256, C_OUT + 1] = w0h
    WE[256, C_OUT + 1] = c0h
    WE[:256, C_OUT + 2] = w1h
    WE[256, C_OUT + 2] = c1h

    XT1h = XT1.astype(np.float16)
    WEh = WE.astype(np.float16)
    A16 = np.asarray(adj_matrix, dtype=np.float16)

    in_maps = []
    for c in range(NCORES):
        in_maps.append(
            {
                "xt1": XT1h,
                "xt1l": np.ascontiguousarray(XT1h[:, c * ROWS : (c + 1) * ROWS]),
                "wext": WEh,
                "adj": np.ascontiguousarray(A16[c * ROWS : (c + 1) * ROWS, :]),
            }
        )
    return in_maps


def _run(in_maps, trace=False, b_zero=True):
    key = f"nc_b{int(b_zero)}"
    if key not in _CACHE:
        _CACHE[key] = _build_nc(b_zero=b_zero)
    nc = _CACHE[key]
    res = run_bass_kernel_spmd(
        nc, in_maps, core_ids=list(range(NCORES)), trace=trace
    )
    full = np.concatenate(
        [res.results[c]["out"] for c in range(NCORES)], axis=0
    ).astype(np.float32)
    return full, res


def kernel(node_feats, adj_matrix, W, b, v0, v1):
    in_maps = _prep_inputs(node_feats, adj_matrix, W, b, v0, v1)
    trace = bool(int(os.environ.get("GAT_TRACE", "0")))
    b_zero = not bool(np.any(np.asarray(b)))
    full, _ = _run(in_maps, trace=trace, b_zero=b_zero)
    return full


# revision 27
# speedup vs baseline: 1.5180x; 1.0553x over previous
"""GAT single-head forward on 8 Trainium2 NeuronCores (Bass/Tile).

Math (per reference):
    h   = X @ W + b                      [N, 128]
    f1  = h @ v0, f2 = h @ v1            [N]
    logits = adj * (f1[:,None] + f2[None,:])   (adj entries are exactly 0/1)
    vals = sigmoid(logits) - 0.5
    masked softmax over row edges; out = probs @ h

Key identities used on device:
  * On edges (adj==1): val = sigmoid(f1_i + f2_j) - 0.5 in (-0.5, 0.5), so the
    softmax max-subtraction is unnecessary (exp never overflows) and constant
    shifts cancel:  probs = adj*exp(sigmoid(s)) / rowsum(adj*exp(sigmoid(s))).
  * sigmoid(s) = 0.5*tanh(s/2) + 0.5, so exp(sigmoid(s)) = exp(0.5*t + 0.5)
    with t = tanh(s/2). Tanh and Exp live in the SAME activation table set
    ("exp_and_others"), avoiding per-tile ~2.7us table swaps that
    Sigmoid+Exp would incur.
  * A ones-column appended to h turns the softmax denominator into one extra
    matmul output column (no separate row-reduction pass).

Sharding: rows of adj across the 8 cores (1024 rows each). node_feats is
small (8 MB) and is replicated, so every core computes the full projected
h locally - no collectives at all.

Per-core layout trick: each core works on its adj block TRANSPOSED
([j=source node on partitions, i=own rows on free dim]) so that the
aggregate probs@h contracts over the partition dim as the tensor engine
requires. The transpose rides the DMA x-bar; adj is cast to fp16 host-side
(exact for a 0/1 mask, and halves HBM traffic).

The projection inputs are fed in fp16 (validated 4.2e-4 scale-relative
absmax on the final output): fp16 matmuls stream 2x faster and get fast
weight loads, and the w0/w1 columns are pre-halved on the host so the
tanh argument s/2 comes straight out of the projection matmul.

Schedule shape (engines are in-order; emission order seeds the queues):
  preamble -> [weights/features DMA | f1 path] -> h-projection batches,
  with the first activation groups' prep (adj transpose DMA, s=f1+f2,
  tanh, exp) interleaved as soon as their h batches drain -> steady
  pipeline: ACT runs tanh/exp back-to-back; DVE preadds+mask-muls; PE
  aggregates into 8 PSUM accumulators -> epilogue (denominator divide)
  and one batched output DMA.
"""

import os

import numpy as np

import concourse.bass as bass
import concourse.mybir as mybir
import concourse.tile as tile
from concourse import bacc
from concourse.bass_utils import run_bass_kernel_spmd

F32 = mybir.dt.float32
F16 = mybir.dt.float16
AF = mybir.ActivationFunctionType

N, C_IN, C_OUT = 8192, 256, 128
NCORES = 8
ROWS = N // NCORES          # 1024 rows of adj per core
P = 128
NT = N // P                 # 64 node tiles (also the j-chunks)
NI = ROWS // P              # 8 output row-tiles per core
KC = [128, 128, 1]          # contraction chunks of K=257 (X.T rows + ones row)
WCOLS = C_OUT + 3           # [W | ones-hack | 0.5*w0 | 0.5*w1]
HCOLS = C_OUT + 1           # h plus the ones column
TINY = float(np.finfo(np.float32).tiny)
BANK = 512                  # PSUM bank, fp32 elements

# activation groups: j-chunks fused per tanh/exp instruction. The first two
# are small so the activation chain starts as early as possible (they only
# need the first h-projection batches); the last are small so the post-exp
# tail (mask-mul + matmul + epilogue) is short.
GROUPS = [4, 4] + [8] * 6 + [4, 2, 2]

_CACHE: dict = {}


def _build_nc(b_zero=True):
    nc = bacc.Bacc(
        "TRN2", target_bir_lowering=False, debug=False, num_devices=NCORES
    )
    xt1 = nc.dram_tensor("xt1", [257, N], F16, kind="ExternalInput").ap()
    xt1l = nc.dram_tensor("xt1l", [257, ROWS], F16, kind="ExternalInput").ap()
    wext = nc.dram_tensor("wext", [257, WCOLS], F16, kind="ExternalInput").ap()
    adj = nc.dram_tensor("adj", [ROWS, N], F16, kind="ExternalInput").ap()
    out = nc.dram_tensor("out", [ROWS, C_OUT], F32, kind="ExternalOutput").ap()

    with tile.TileContext(nc) as tc:
        _emit(tc, nc, xt1, xt1l, wext, adj, out, b_zero)
    nc.compile()
    return nc


def _emit(tc, nc, xt1, xt1l, wext, adj, out, b_zero):
    from contextlib import ExitStack

    # with b == 0 the K=1 "ones row" contraction chunk only contributes the
    # constant-one column of h_ext (done with a strided memset instead) and
    # zero constants to f1/f2 -- skip it entirely.
    nkc = 2 if b_zero else 3

    with ExitStack() as ctx:
        # ---- persistent tiles ----
        persist = ctx.enter_context(tc.tile_pool(name="persist", bufs=1))
        h16_all = persist.tile([P, NT * HCOLS], F16, tag="h16")   # [128, 8256]
        f2h_all = persist.tile([P, NT], F32, tag="f2h")           # 0.5*f2 per j
        f1rep = persist.tile([P, ROWS], F16, tag="f1rep")         # f1/2 bcast
        ones_row = persist.tile([1, P], F16, tag="ones")
        nc.vector.memset(ones_row[:], 1.0)
        zero1 = persist.tile([P, 1], F32, tag="zero1")
        nc.vector.memset(zero1[:], 0.0)
        half1 = persist.tile([P, 1], F32, tag="half1")
        nc.vector.memset(half1[:], 0.5)
        if b_zero:
            # constant-one column of every h_ext tile (replaces the K=1
            # bias matmul chunk)
            nc.vector.memset(
                h16_all[:].rearrange("p (t c) -> p t c", c=HCOLS)[
                    :, :, C_OUT : C_OUT + 1
                ],
                1.0,
            )

        xtp = ctx.enter_context(tc.tile_pool(name="xt", bufs=1))
        fsb = ctx.enter_context(tc.tile_pool(name="fsb", bufs=1))

        # ---- input loads ----
        # small inputs first so the f1 path clears quickly. The xt sub-loads
        # are interleaved k0/k1 so the first node tiles have BOTH
        # contraction chunks resident as early as possible (tile dependency
        # tracking is AP-range based).
        wes, xls = [], []
        off = 0
        for k in range(nkc):
            kc = KC[k]
            we_sb = xtp.tile([kc, WCOLS], F16, tag=f"we{k}")
            nc.sync.dma_start(we_sb[:], wext[off : off + kc, :])
            xl_sb = xtp.tile([kc, ROWS], F16, tag=f"xl{k}")
            nc.sync.dma_start(xl_sb[:], xt1l[off : off + kc, :])
            wes.append(we_sb)
            xls.append(xl_sb)
            off += kc
        offs = [0, 128, 256]
        xts = [
            xtp.tile([KC[k], N], F16, name=f"xtsb{k}", tag=f"xt{k}")
            for k in range(nkc)
        ]
        NSUB = 4
        sub = N // NSUB
        for c in range(NSUB):
            for k in range(nkc):
                if KC[k] != P:
                    if c == 0:
                        nc.sync.dma_start(
                            xts[k][:], xt1[offs[k] : offs[k] + KC[k], :]
                        )
                    continue
                nc.sync.dma_start(
                    xts[k][:, c * sub : (c + 1) * sub],
                    xt1[offs[k] : offs[k] + KC[k], c * sub : (c + 1) * sub],
                )

        # ---- f1 path: f1/2 for this core's rows, broadcast to all parts ----
        with tc.tile_pool(name="pf", bufs=1, space="PSUM") as pfp:
            pf1 = pfp.tile([1, ROWS], F32, tag="pf1")
            for k in range(nkc):
                for nh in range(ROWS // 512):
                    nc.tensor.matmul(
                        pf1[:, nh * 512 : (nh + 1) * 512],
                        wes[k][:, C_OUT + 1 : C_OUT + 2],
                        xls[k][:, nh * 512 : (nh + 1) * 512],
                        start=(k == 0),
                        stop=(k == nkc - 1),
                    )
            f1row = fsb.tile([1, ROWS], F16, tag="f1row")
            nc.scalar.copy(f1row[:], pf1[:])

            # broadcast across partitions via a K=1 matmul with a ones col
            prep = pfp.tile([P, ROWS], F32, tag="prep")
            for nh in range(ROWS // 512):
                sl = slice(nh * 512, (nh + 1) * 512)
                nc.tensor.matmul(
                    prep[:, sl], ones_row[:], f1row[:, sl],
                    start=True, stop=True,
                )
            nc.scalar.copy(f1rep[:], prep[:])

        # ---- main-loop pools (open before the h loop so activation groups
        # can be emitted interleaved with h batches) ----
        sup = ctx.enter_context(tc.tile_pool(name="sup", bufs=1))
        g16p = ctx.enter_context(tc.tile_pool(name="g16p", bufs=2))
        atp = ctx.enter_context(tc.tile_pool(name="atp", bufs=14))
        etp = ctx.enter_context(tc.tile_pool(name="etp", bufs=8))
        obp = ctx.enter_context(tc.tile_pool(name="ob", bufs=2))

        group_q0 = []
        q0 = 0
        for gsz in GROUPS:
            group_q0.append(q0)
            q0 += gsz

        deferred = []  # groups whose mask-mul+matmul emission is pending

        def emit_group_front(g):
            """adj transposes, s=f1+f2 preadds, fused tanh, fused exp."""
            gsz = GROUPS[g]
            q0 = group_q0[g]
            s_sup = sup.tile([P, gsz * ROWS], F16, tag="s", bufs=2, name=f"s{g}")
            t_sup = sup.tile([P, gsz * ROWS], F16, tag="t", bufs=1, name=f"t{g}")
            g16 = g16p.tile([P, gsz * ROWS], F16, tag="g16", name=f"g16_{g}")
            ats = []
            for qq in range(gsz):
                q = q0 + qq
                at = atp.tile([P, ROWS], F16, tag="at", name=f"at{q}")
                nc.sync.dma_start(
                    at[:], adj[:, q * P : (q + 1) * P], transpose=True
                )
                ats.append(at)
                # s = 0.5*f1_i + 0.5*f2_j   [j on partitions, i on free]
                nc.vector.tensor_scalar_add(
                    s_sup[:, qq * ROWS : (qq + 1) * ROWS],
                    f1rep[:],
                    f2h_all[:, q : q + 1],
                )
            nc.scalar.activation(t_sup[:], s_sup[:], AF.Tanh, bias=zero1[:])
            # exp(0.5*tanh + 0.5) = exp(sigmoid(s)), output fp16
            nc.scalar.activation(
                g16[:], t_sup[:], AF.Exp, bias=half1[:], scale=0.5
            )
            return {"g": g, "gsz": gsz, "q0": q0, "ats": ats, "g16": g16}

        def emit_group_back(fr, pouts):
            """mask-mul + aggregate matmuls for a prepared group."""
            gsz, q0, ats, g16 = fr["gsz"], fr["q0"], fr["ats"], fr["g16"]
            for qq in range(gsz):
                q = q0 + qq
                et = etp.tile([P, ROWS], F16, tag="et", name=f"et{q}")
                nc.vector.tensor_mul(
                    et[:], ats[qq][:], g16[:, qq * ROWS : (qq + 1) * ROWS]
                )
                rhs = h16_all[:, q * HCOLS : (q + 1) * HCOLS]
                for it in range(NI):
                    nc.tensor.matmul(
                        pouts[it][:],
                        et[:, it * P : (it + 1) * P],
                        rhs,
                        start=(q == 0),
                        stop=(q == NT - 1),
                    )

        # ---- h-projection: all 8 PSUM banks inside ONE tensor so four
        # tiles drain with a single strided copy. Pairs of node tiles have
        # their k-chunk matmuls interleaved so consecutive matmuls hit
        # different banks (same-bank accumulation serializes the PE). ----
        next_group = 0
        with tc.tile_pool(name="php", bufs=1, space="PSUM") as php:
            ph_all = php.tile([P, NI * BANK], F32, tag="ph")
            for b in range(NT // 4):  # batches of 4 node tiles
                for half in range(2):
                    nt0 = 4 * b + 2 * half
                    w0 = (nt0 % NI) * BANK
                    w1 = ((nt0 + 1) % NI) * BANK
                    for k in range(nkc):
                        nc.tensor.matmul(
                            ph_all[:, w0 : w0 + WCOLS],
                            xts[k][:, nt0 * P : (nt0 + 1) * P],
                            wes[k][:],
                            start=(k == 0),
                            stop=(k == nkc - 1),
                        )
                        nc.tensor.matmul(
                            ph_all[:, w1 : w1 + WCOLS],
                            xts[k][:, (nt0 + 1) * P : (nt0 + 2) * P],
                            wes[k][:],
                            start=(k == 0),
                            stop=(k == nkc - 1),
                        )
                # drain the 4 fresh tiles: h (+ones col) -> fp16, 0.5*f2 col
                bt = 4 * b
                wlo = (bt % NI) * BANK
                src = ph_all[:, wlo : wlo + 4 * BANK].rearrange(
                    "p (b w) -> p b w", b=4
                )
                dst_h = h16_all[:, bt * HCOLS : (bt + 4) * HCOLS].rearrange(
                    "p (b w) -> p b w", b=4
                )
                hc = C_OUT if b_zero else HCOLS
                nc.vector.tensor_copy(dst_h[:, :, 0:hc], src[:, :, 0:hc])
                nc.vector.tensor_copy(
                    f2h_all[:, bt : bt + 4], src[:, :, C_OUT + 2 : C_OUT + 3]
                )
                # emit activation-group fronts as soon as their f2 columns
                # exist; their matmuls wait until the PSUM banks free up
                while (
                    next_group < len(GROUPS)
                    and group_q0[next_group] + GROUPS[next_group] <= 4 * (b + 1)
                    and len(deferred) < 3
                ):
                    deferred.append(emit_group_front(next_group))
                    next_group += 1

        # ---- aggregate accumulators: same 8 banks, next accumulation ----
        pop = ctx.enter_context(tc.tile_pool(name="po", bufs=1, space="PSUM"))
        pouts = [
            pop.tile([P, HCOLS], F32, name=f"po{i}", tag=f"po{i}")
            for i in range(NI)
        ]

        for fr in deferred:
            emit_group_back(fr, pouts)
        for g in range(next_group, len(GROUPS)):
            fr = emit_group_front(g)
            emit_group_back(fr, pouts)

        # ---- epilogue: divide by clamped denominator, one batched store ----
        ob_all = obp.tile([P, NI * C_OUT], F32, tag="oball")
        for it in range(NI):
            po = pouts[it]
            dm = obp.tile([P, 1], F32, tag="dm", bufs=4, name=f"dm{it}")
            nc.vector.tensor_scalar_max(dm[:], po[:, C_OUT : C_OUT + 1], TINY)
            rc = obp.tile([P, 1], F32, tag="rc", bufs=4, name=f"rc{it}")
            nc.vector.reciprocal(rc[:], dm[:])
            nc.vector.tensor_scalar_mul(
                ob_all[:, it * C_OUT : (it + 1) * C_OUT], po[:, 0:C_OUT], rc[:]
            )
        nc.sync.dma_start(
            out.rearrange("(t p) c -> p t c", p=P),
            ob_all[:].rearrange("p (t c) -> p t c", c=C_OUT),
        )


def _prep_inputs(node_feats, adj_matrix, W, b, v0, v1):
    X = np.ascontiguousarray(node_feats, dtype=np.float32)
    W = np.asarray(W, dtype=np.float32)
    b = np.asarray(b, dtype=np.float32)
    v0 = np.asarray(v0, dtype=np.float32)
    v1 = np.asarray(v1, dtype=np.float32)

    w0h = (0.5 * (W.astype(np.float64) @ v0.astype(np.float64))).astype(np.float32)
    w1h = (0.5 * (W.astype(np.float64) @ v1.astype(np.float64))).astype(np.float32)
    c0h = np.float32(0.5 * float(b.astype(np.float64) @ v0.astype(np.float64)))
    c1h = np.float32(0.5 * float(b.astype(np.float64) @ v1.astype(np.float64)))

    XT1 = np.empty((257, N), np.float32)
    XT1[:256] = X.T
    XT1[256] = 1.0

    WE = np.zeros((257, WCOLS), np.float32)
    WE[:256, :C_OUT] = W
    WE[256, :C_OUT] = b
    WE[256, C_OUT] = 1.0          # makes h_ext column 128 identically 1
    WE[:256, C_OUT + 1] = w0h
    WE[256, C_OUT + 1] = c0h
    WE[:256, C_OUT + 2] = w1h
    WE[256, C_OUT + 2] = c1h

    XT1h = XT1.astype(np.float16)
    WEh = WE.astype(np.float16)
    A16 = np.asarray(adj_matrix, dtype=np.float16)

    in_maps = []
    for c in range(NCORES):
        in_maps.append(
            {
                "xt1": XT1h,
                "xt1l": np.ascontiguousarray(XT1h[:, c * ROWS : (c + 1) * ROWS]),
                "wext": WEh,
                "adj": np.ascontiguousarray(A16[c * ROWS : (c + 1) * ROWS, :]),
            }
        )
    return in_maps


def _run(in_maps, trace=False, b_zero=True):
    key = f"nc_b{int(b_zero)}"
    if key not in _CACHE:
        _CACHE[key] = _build_nc(b_zero=b_zero)
    nc = _CACHE[key]
    res = run_bass_kernel_spmd(
        nc, in_maps, core_ids=list(range(NCORES)), trace=trace
    )
    full = np.concatenate(
        [res.results[c]["out"] for c in range(NCORES)], axis=0
    ).astype(np.float32)
    return full, res


def kernel(node_feats, adj_matrix, W, b, v0, v1):
    in_maps = _prep_inputs(node_feats, adj_matrix, W, b, v0, v1)
    trace = bool(int(os.environ.get("GAT_TRACE", "0")))
    b_zero = not bool(np.any(np.asarray(b)))
    full, _ = _run(in_maps, trace=trace, b_zero=b_zero)
    return full


# revision 28
# speedup vs baseline: 1.5326x; 1.0096x over previous
"""GAT single-head forward on 8 Trainium2 NeuronCores (Bass/Tile).

Math (per reference):
    h   = X @ W + b                      [N, 128]
    f1  = h @ v0, f2 = h @ v1            [N]
    logits = adj * (f1[:,None] + f2[None,:])   (adj entries are exactly 0/1)
    vals = sigmoid(logits) - 0.5
    masked softmax over row edges; out = probs @ h

Key identities used on device:
  * On edges (adj==1): val = sigmoid(f1_i + f2_j) - 0.5 in (-0.5, 0.5), so the
    softmax max-subtraction is unnecessary (exp never overflows) and constant
    shifts cancel:  probs = adj*exp(sigmoid(s)) / rowsum(adj*exp(sigmoid(s))).
  * sigmoid(s) = 0.5*tanh(s/2) + 0.5, so exp(sigmoid(s)) = exp(0.5*t + 0.5)
    with t = tanh(s/2). Tanh and Exp live in the SAME activation table set
    ("exp_and_others"), avoiding per-tile ~2.7us table swaps that
    Sigmoid+Exp would incur.
  * A ones-column appended to h turns the softmax denominator into one extra
    matmul output column (no separate row-reduction pass).

Sharding: rows of adj across the 8 cores (1024 rows each). node_feats is
small (8 MB) and is replicated, so every core computes the full projected
h locally - no collectives at all.

Per-core layout trick: each core works on its adj block TRANSPOSED
([j=source node on partitions, i=own rows on free dim]) so that the
aggregate probs@h contracts over the partition dim as the tensor engine
requires. The transpose rides the DMA x-bar; adj is cast to fp16 host-side
(exact for a 0/1 mask, and halves HBM traffic).

The projection inputs are fed in fp16 (validated 4.2e-4 scale-relative
absmax on the final output): fp16 matmuls stream 2x faster and get fast
weight loads, and the w0/w1 columns are pre-halved on the host so the
tanh argument s/2 comes straight out of the projection matmul.

Schedule shape (engines are in-order; emission order seeds the queues):
  preamble -> [weights/features DMA | f1 path] -> h-projection batches,
  with the first activation groups' prep (adj transpose DMA, s=f1+f2,
  tanh, exp) interleaved as soon as their h batches drain -> steady
  pipeline: ACT runs tanh/exp back-to-back; DVE preadds+mask-muls; PE
  aggregates into 8 PSUM accumulators -> epilogue (denominator divide)
  and one batched output DMA.
"""

import os

import numpy as np

import concourse.bass as bass
import concourse.mybir as mybir
import concourse.tile as tile
from concourse import bacc
from concourse.bass_utils import run_bass_kernel_spmd

F32 = mybir.dt.float32
F16 = mybir.dt.float16
AF = mybir.ActivationFunctionType

N, C_IN, C_OUT = 8192, 256, 128
NCORES = 8
ROWS = N // NCORES          # 1024 rows of adj per core
P = 128
NT = N // P                 # 64 node tiles (also the j-chunks)
NI = ROWS // P              # 8 output row-tiles per core
KC = [128, 128, 1]          # contraction chunks of K=257 (X.T rows + ones row)
WCOLS = C_OUT + 3           # [W | ones-hack | 0.5*w0 | 0.5*w1]
HCOLS = C_OUT + 1           # h plus the ones column
TINY = float(np.finfo(np.float32).tiny)
BANK = 512                  # PSUM bank, fp32 elements

# activation groups: j-chunks fused per tanh/exp instruction. The first two
# are small so the activation chain starts as early as possible (they only
# need the first h-projection batches); the last are small so the post-exp
# tail (mask-mul + matmul + epilogue) is short.
GROUPS = [4, 4] + [8] * 6 + [4, 2, 2]

_CACHE: dict = {}


def _build_nc(b_zero=True):
    nc = bacc.Bacc(
        "TRN2", target_bir_lowering=False, debug=False, num_devices=NCORES
    )
    xt1 = nc.dram_tensor("xt1", [257, N], F16, kind="ExternalInput").ap()
    xt1l = nc.dram_tensor("xt1l", [257, ROWS], F16, kind="ExternalInput").ap()
    wext = nc.dram_tensor("wext", [257, WCOLS], F16, kind="ExternalInput").ap()
    adjt = nc.dram_tensor("adjt", [N, ROWS], F16, kind="ExternalInput").ap()
    out = nc.dram_tensor("out", [ROWS, C_OUT], F32, kind="ExternalOutput").ap()

    with tile.TileContext(nc) as tc:
        _emit(tc, nc, xt1, xt1l, wext, adjt, out, b_zero)
    nc.compile()
    return nc


def _emit(tc, nc, xt1, xt1l, wext, adjt, out, b_zero):
    from contextlib import ExitStack

    # with b == 0 the K=1 "ones row" contraction chunk only contributes the
    # constant-one column of h_ext (done with a strided memset instead) and
    # zero constants to f1/f2 -- skip it entirely.
    nkc = 2 if b_zero else 3

    with ExitStack() as ctx:
        # ---- persistent tiles ----
        persist = ctx.enter_context(tc.tile_pool(name="persist", bufs=1))
        h16_all = persist.tile([P, NT * HCOLS], F16, tag="h16")   # [128, 8256]
        f2h_all = persist.tile([P, NT], F32, tag="f2h")           # 0.5*f2 per j
        f1rep = persist.tile([P, ROWS], F16, tag="f1rep")         # f1/2 bcast
        ones_row = persist.tile([1, P], F16, tag="ones")
        nc.vector.memset(ones_row[:], 1.0)
        zero1 = persist.tile([P, 1], F32, tag="zero1")
        nc.vector.memset(zero1[:], 0.0)
        half1 = persist.tile([P, 1], F32, tag="half1")
        nc.vector.memset(half1[:], 0.5)
        if b_zero:
            # constant-one column of every h_ext tile (replaces the K=1
            # bias matmul chunk)
            nc.vector.memset(
                h16_all[:].rearrange("p (t c) -> p t c", c=HCOLS)[
                    :, :, C_OUT : C_OUT + 1
                ],
                1.0,
            )

        xtp = ctx.enter_context(tc.tile_pool(name="xt", bufs=1))
        fsb = ctx.enter_context(tc.tile_pool(name="fsb", bufs=1))

        # ---- input loads ----
        # small inputs first so the f1 path clears quickly. The xt sub-loads
        # are interleaved k0/k1 so the first node tiles have BOTH
        # contraction chunks resident as early as possible (tile dependency
        # tracking is AP-range based).
        wes, xls = [], []
        off = 0
        for k in range(nkc):
            kc = KC[k]
            wx_sb = xtp.tile([kc, WCOLS + ROWS], F16, name=f"wx{k}", tag=f"wx{k}")
            nc.sync.dma_start(wx_sb[:, 0:WCOLS], wext[off : off + kc, :])
            nc.sync.dma_start(wx_sb[:, WCOLS:], xt1l[off : off + kc, :])
            wes.append(wx_sb[:, 0:WCOLS])
            xls.append(wx_sb[:, WCOLS:])
            off += kc
        offs = [0, 128, 256]
        xts = [
            xtp.tile([KC[k], N], F16, name=f"xtsb{k}", tag=f"xt{k}")
            for k in range(nkc)
        ]
        NSUB = 4
        sub = N // NSUB
        for c in range(NSUB):
            for k in range(nkc):
                if KC[k] != P:
                    if c == 0:
                        nc.sync.dma_start(
                            xts[k][:], xt1[offs[k] : offs[k] + KC[k], :]
                        )
                    continue
                nc.sync.dma_start(
                    xts[k][:, c * sub : (c + 1) * sub],
                    xt1[offs[k] : offs[k] + KC[k], c * sub : (c + 1) * sub],
                )

        # ---- f1 path: f1/2 for this core's rows, broadcast to all parts ----
        with tc.tile_pool(name="pf", bufs=1, space="PSUM") as pfp:
            pf1 = pfp.tile([1, ROWS], F32, tag="pf1")
            for k in range(nkc):
                for nh in range(ROWS // 512):
                    nc.tensor.matmul(
                        pf1[:, nh * 512 : (nh + 1) * 512],
                        wes[k][:, C_OUT + 1 : C_OUT + 2],
                        xls[k][:, nh * 512 : (nh + 1) * 512],
                        start=(k == 0),
                        stop=(k == nkc - 1),
                    )
            f1row = fsb.tile([1, ROWS], F16, tag="f1row")
            nc.scalar.copy(f1row[:], pf1[:])

            # broadcast across partitions via a K=1 matmul with a ones col
            prep = pfp.tile([P, ROWS], F32, tag="prep")
            for nh in range(ROWS // 512):
                sl = slice(nh * 512, (nh + 1) * 512)
                nc.tensor.matmul(
                    prep[:, sl], ones_row[:], f1row[:, sl],
                    start=True, stop=True,
                )
            nc.scalar.copy(f1rep[:], prep[:])

        # ---- main-loop pools (open before the h loop so activation groups
        # can be emitted interleaved with h batches) ----
        sup = ctx.enter_context(tc.tile_pool(name="sup", bufs=1))
        g16p = ctx.enter_context(tc.tile_pool(name="g16p", bufs=2))
        atp = ctx.enter_context(tc.tile_pool(name="atp", bufs=2))
        etp = ctx.enter_context(tc.tile_pool(name="etp", bufs=6))
        obp = ctx.enter_context(tc.tile_pool(name="ob", bufs=2))

        group_q0 = []
        q0 = 0
        for gsz in GROUPS:
            group_q0.append(q0)
            q0 += gsz

        deferred = []  # groups whose mask-mul+matmul emission is pending

        def emit_group_front(g):
            """adj transposes, s=f1+f2 preadds, fused tanh, fused exp."""
            gsz = GROUPS[g]
            q0 = group_q0[g]
            s_sup = sup.tile([P, gsz * ROWS], F16, tag="s", bufs=2, name=f"s{g}")
            t_sup = sup.tile([P, gsz * ROWS], F16, tag="t", bufs=1, name=f"t{g}")
            g16 = g16p.tile([P, gsz * ROWS], F16, tag="g16", name=f"g16_{g}")
            at_sup = atp.tile(
                [P, gsz * ROWS], F16, tag="at", name=f"at{g}"
            )
            nc.sync.dma_start(
                at_sup[:].rearrange("p (q i) -> p q i", i=ROWS),
                adjt.rearrange("(q p) i -> p q i", p=P)[:, q0 : q0 + gsz, :],
            )
            for qq in range(gsz):
                q = q0 + qq
                # s = 0.5*f1_i + 0.5*f2_j   [j on partitions, i on free]
                nc.vector.tensor_scalar_add(
                    s_sup[:, qq * ROWS : (qq + 1) * ROWS],
                    f1rep[:],
                    f2h_all[:, q : q + 1],
                )
            nc.scalar.activation(t_sup[:], s_sup[:], AF.Tanh, bias=zero1[:])
            # exp(0.5*tanh + 0.5) = exp(sigmoid(s)), output fp16
            nc.scalar.activation(
                g16[:], t_sup[:], AF.Exp, bias=half1[:], scale=0.5
            )
            return {"g": g, "gsz": gsz, "q0": q0, "at": at_sup, "g16": g16}

        def emit_group_back(fr, pouts):
            """mask-mul + aggregate matmuls for a prepared group."""
            gsz, q0, at_sup, g16 = fr["gsz"], fr["q0"], fr["at"], fr["g16"]
            for qq in range(gsz):
                q = q0 + qq
                et = etp.tile([P, ROWS], F16, tag="et", name=f"et{q}")
                nc.vector.tensor_mul(
                    et[:],
                    at_sup[:, qq * ROWS : (qq + 1) * ROWS],
                    g16[:, qq * ROWS : (qq + 1) * ROWS],
                )
                rhs = h16_all[:, q * HCOLS : (q + 1) * HCOLS]
                for it in range(NI):
                    nc.tensor.matmul(
                        pouts[it],
                        et[:, it * P : (it + 1) * P],
                        rhs,
                        start=(q == 0),
                        stop=(q == NT - 1),
                    )

        # ---- h-projection: all 8 PSUM banks inside ONE tensor so four
        # tiles drain with a single strided copy. Pairs of node tiles have
        # their k-chunk matmuls interleaved so consecutive matmuls hit
        # different banks (same-bank accumulation serializes the PE). ----
        next_group = 0
        with tc.tile_pool(name="php", bufs=1, space="PSUM") as php:
            ph_all = php.tile([P, NI * BANK], F32, tag="ph")
            for b in range(NT // 4):  # batches of 4 node tiles
                for half in range(2):
                    nt0 = 4 * b + 2 * half
                    w0 = (nt0 % NI) * BANK
                    w1 = ((nt0 + 1) % NI) * BANK
                    for k in range(nkc):
                        nc.tensor.matmul(
                            ph_all[:, w0 : w0 + WCOLS],
                            xts[k][:, nt0 * P : (nt0 + 1) * P],
                            wes[k][:],
                            start=(k == 0),
                            stop=(k == nkc - 1),
                        )
                        nc.tensor.matmul(
                            ph_all[:, w1 : w1 + WCOLS],
                            xts[k][:, (nt0 + 1) * P : (nt0 + 2) * P],
                            wes[k][:],
                            start=(k == 0),
                            stop=(k == nkc - 1),
                        )
                # drain the 4 fresh tiles: h (+ones col) -> fp16, 0.5*f2 col
                bt = 4 * b
                wlo = (bt % NI) * BANK
                src = ph_all[:, wlo : wlo + 4 * BANK].rearrange(
                    "p (b w) -> p b w", b=4
                )
                dst_h = h16_all[:, bt * HCOLS : (bt + 4) * HCOLS].rearrange(
                    "p (b w) -> p b w", b=4
                )
                hc = C_OUT if b_zero else HCOLS
                nc.vector.tensor_copy(dst_h[:, :, 0:hc], src[:, :, 0:hc])
                nc.vector.tensor_copy(
                    f2h_all[:, bt : bt + 4], src[:, :, C_OUT + 2 : C_OUT + 3]
                )
                # emit activation-group fronts as soon as their f2 columns
                # exist; their matmuls wait until the PSUM banks free up
                while (
                    next_group < len(GROUPS)
                    and group_q0[next_group] + GROUPS[next_group] <= 4 * (b + 1)
                    and len(deferred) < 3
                ):
                    deferred.append(emit_group_front(next_group))
                    next_group += 1

        # ---- aggregate accumulators: same 8 banks, next accumulation ----
        pop = ctx.enter_context(tc.tile_pool(name="po", bufs=1, space="PSUM"))
        po_all = pop.tile([P, NI * BANK], F32, tag="poall")
        pouts = [po_all[:, i * BANK : i * BANK + HCOLS] for i in range(NI)]

        for fr in deferred:
            emit_group_back(fr, pouts)
        for g in range(next_group, len(GROUPS)):
            fr = emit_group_front(g)
            emit_group_back(fr, pouts)

        # ---- epilogue: divide by clamped denominator, one batched store ----
        ob_all = obp.tile([P, NI * C_OUT], F32, tag="oball")
        po3 = po_all[:].rearrange("p (t w) -> p t w", w=BANK)
        dm = obp.tile([P, NI], F32, tag="dm")
        nc.vector.tensor_scalar_max(
            dm[:], po3[:, :, C_OUT : C_OUT + 1], TINY
        )
        rc = obp.tile([P, NI], F32, tag="rc")
        nc.vector.reciprocal(rc[:], dm[:])
        for it in range(NI):
            nc.vector.tensor_scalar_mul(
                ob_all[:, it * C_OUT : (it + 1) * C_OUT],
                po_all[:, it * BANK : it * BANK + C_OUT],
                rc[:, it : it + 1],
            )
        nc.sync.dma_start(
            out.rearrange("(t p) c -> p t c", p=P),
            ob_all[:].rearrange("p (t c) -> p t c", c=C_OUT),
        )


def _prep_inputs(node_feats, adj_matrix, W, b, v0, v1):
    X = np.ascontiguousarray(node_feats, dtype=np.float32)
    W = np.asarray(W, dtype=np.float32)
    b = np.asarray(b, dtype=np.float32)
    v0 = np.asarray(v0, dtype=np.float32)
    v1 = np.asarray(v1, dtype=np.float32)

    w0h = (0.5 * (W.astype(np.float64) @ v0.astype(np.float64))).astype(np.float32)
    w1h = (0.5 * (W.astype(np.float64) @ v1.astype(np.float64))).astype(np.float32)
    c0h = np.float32(0.5 * float(b.astype(np.float64) @ v0.astype(np.float64)))
    c1h = np.float32(0.5 * float(b.astype(np.float64) @ v1.astype(np.float64)))

    XT1 = np.empty((257, N), np.float32)
    XT1[:256] = X.T
    XT1[256] = 1.0

    WE = np.zeros((257, WCOLS), np.float32)
    WE[:256, :C_OUT] = W
    WE[256, :C_OUT] = b
    WE[256, C_OUT] = 1.0          # makes h_ext column 128 identically 1
    WE[:256, C_OUT + 1] = w0h
    WE[256, C_OUT + 1] = c0h
    WE[:256, C_OUT + 2] = w1h
    WE[256, C_OUT + 2] = c1h

    XT1h = XT1.astype(np.float16)
    WEh = WE.astype(np.float16)
    A16 = np.asarray(adj_matrix, dtype=np.float16)

    in_maps = []
    for c in range(NCORES):
        in_maps.append(
            {
                "xt1": XT1h,
                "xt1l": np.ascontiguousarray(XT1h[:, c * ROWS : (c + 1) * ROWS]),
                "wext": WEh,
                "adjt": np.ascontiguousarray(
                    A16[c * ROWS : (c + 1) * ROWS, :].T
                ),
            }
        )
    return in_maps


def _run(in_maps, trace=False, b_zero=True):
    key = f"nc_b{int(b_zero)}"
    if key not in _CACHE:
        _CACHE[key] = _build_nc(b_zero=b_zero)
    nc = _CACHE[key]
    res = run_bass_kernel_spmd(
        nc, in_maps, core_ids=list(range(NCORES)), trace=trace
    )
    full = np.concatenate(
        [res.results[c]["out"] for c in range(NCORES)], axis=0
    ).astype(np.float32)
    return full, res


def kernel(node_feats, adj_matrix, W, b, v0, v1):
    in_maps = _prep_inputs(node_feats, adj_matrix, W, b, v0, v1)
    trace = bool(int(os.environ.get("GAT_TRACE", "0")))
    b_zero = not bool(np.any(np.asarray(b)))
    full, _ = _run(in_maps, trace=trace, b_zero=b_zero)
    return full
